# revision 43
# baseline (speedup 1.0000x reference)
"""CBAM (channel + spatial attention) Trainium2 kernel, 8-core data parallel.

Problem: f [8, 8, 256, 56, 56] f32 -> out same shape.
  x = f.reshape(BT, C, H, W)
  ca = sigmoid(mlp(max_hw(x)) + mlp(mean_hw(x)));  xc = ca * x
  s  = conv7x7([mean_c(xc); max_c(xc)]);           out = sigmoid(s) * xc

Strategy (per NeuronCore, 8 frames each, no collectives):
  - bf16 end-to-end on device: host casts f to bf16, device returns bf16,
    host upcasts -> halves HBM traffic (rel-err budget 2e-2 >> bf16 noise)
  - loads on SP HWDGE prefetched 2 frames ahead of the frame body so
    mid-chain DMAs never head-of-line-block the next frame's load
  - channel-sum: ACT activation accum_out per tile (f32 accumulator; DVE
    reduce-add accumulates in bf16 on HW and is NOT accurate enough)
  - channel-max: DVE TT fold tree L1-L3 (bf16 2x) + small 1x reduce
  - xc0 via ACT scaled copy (per-partition scale AP), xc1 via DVE
    tensor_scalar (4x); m1 = max(xc0, xc1) via DVE TT (2x)
  - ssum: PE matmuls into PSUM chunks at partitions {0,32,64} x 3 banks
    (matmul PSUM base partition must be 0/32/64), single ACT drain,
    partition-strided DMA scatters rows 0/32/64 into the conv input
  - smax: 28 PE transposes -> PSUM bf16 (2 groups), 2 direct DVE
    reduces (PSUM-in 1x, bf16 out), transpose back, ACT drain
  - conv 7x7 as 7 accumulating PE matmuls (banded lhsT, 1/C folded in)
  - sa broadcast to 128 partitions via GPSIMD partition_broadcast ucode
    (DMA stride-0 broadcast serializes on the source partition; don't)
  - final: ob_t = xc_t * sab via DVE TT (2x); ob emitted one frame late
    (software pipelining); store t0 on GPSIMD SWDGE, t1 on SP HWDGE
"""

import sys
from contextlib import ExitStack

import numpy as np

if "/opt/trn_rl_repo" not in sys.path:
    sys.path.insert(0, "/opt/trn_rl_repo")

import concourse.bass as bass
import concourse.tile as tile
from concourse import bacc, mybir
from concourse.bass_utils import run_bass_kernel_spmd
from concourse.masks import make_identity

F32 = mybir.dt.float32
BF16 = mybir.dt.bfloat16
ALU = mybir.AluOpType
ACTF = mybir.ActivationFunctionType

N_CORES = 8
B, T, C, H, W = 8, 8, 256, 56, 56
HW = H * W            # 3136
FRAMES = B * T        # 64
FPC = FRAMES // N_CORES  # frames per core = 8
PAD = 3
HP, WP = H + 2 * PAD, W + 2 * PAD  # 62, 62
SCHK = 448            # ssum chunk width (7 chunks)
TCHK = 112            # transpose chunk (28 chunks, 2 groups of 14)
HHW = HW // 2         # 1568


def _build_conv_lhsT(conv_w: np.ndarray) -> np.ndarray:
    """Banded matrices for the 7x7 conv as 7 accumulating matmuls over y.

    B[dx] : [124, 56], rows = c*62 + y_in, cols = y_out.
    B[dx][c*62 + yi, yo] = w_eff[c, yi-yo, dx] for 0 <= yi-yo <= 6.
    The channel-mean 1/C is folded into the avg branch (c=0).
    """
    w_eff = conv_w[0].astype(np.float64).copy()  # [2, 7, 7]
    w_eff[0] /= C
    Bm = np.zeros((7, 2 * HP, H), dtype=np.float32)
    dyi = np.arange(7)
    for dx in range(7):
        for c in range(2):
            for yo in range(H):
                Bm[dx, c * HP + yo + dyi, yo] = w_eff[c, :, dx]
    return Bm


def build_nc(n_frames: int = FPC):
    nc = bacc.Bacc("TRN2", target_bir_lowering=False, debug=False,
                   num_devices=N_CORES)

    x_ext = nc.dram_tensor("x", [n_frames, C, HW], BF16, kind="ExternalInput")
    w1_ext = nc.dram_tensor("w1", [C, 16], F32, kind="ExternalInput")
    w2_ext = nc.dram_tensor("w2", [16, C], F32, kind="ExternalInput")
    cb_ext = nc.dram_tensor("convb", [7, 2 * HP, H], F32, kind="ExternalInput")
    out_ext = nc.dram_tensor("out", [n_frames, C, HW], BF16,
                             kind="ExternalOutput")

    with tile.TileContext(nc) as tc, ExitStack() as ctx:
        consts = ctx.enter_context(tc.tile_pool(name="consts", bufs=1))
        xin = ctx.enter_context(tc.tile_pool(name="xin", bufs=3))
        scrp = ctx.enter_context(tc.tile_pool(name="scr", bufs=1))
        foldp = ctx.enter_context(tc.tile_pool(name="fold", bufs=1))
        xcp = ctx.enter_context(tc.tile_pool(name="xc", bufs=2))
        m1p = ctx.enter_context(tc.tile_pool(name="m1", bufs=2))
        sabp = ctx.enter_context(tc.tile_pool(name="sab", bufs=2))
        obp = ctx.enter_context(tc.tile_pool(name="ob", bufs=2))
        sap = ctx.enter_context(tc.tile_pool(name="sa", bufs=2))
        small = ctx.enter_context(tc.tile_pool(name="small", bufs=3))
        # PSUM: pss 3 banks + pt 2x2 banks + misc 1 bank = 8
        pssp = ctx.enter_context(tc.tile_pool(name="pss", bufs=1, space="PSUM"))
        ptp = ctx.enter_context(tc.tile_pool(name="pt", bufs=1, space="PSUM"))
        pmp = ctx.enter_context(tc.tile_pool(name="pm", bufs=1, space="PSUM"))

        # ---- constants / weights (loaded once) ----
        w1_sb = consts.tile([128, 2, 16], F32)       # [k, ktile, m]
        for t in range(2):
            nc.sync.dma_start(w1_sb[:, t, :], w1_ext[t * 128:(t + 1) * 128, :])
        w2_sb = consts.tile([16, C], F32)
        nc.sync.dma_start(w2_sb[:], w2_ext[:, :])
        cb_sb = consts.tile([124, 7, H], BF16)       # [y_in(+c), dx, y_out]
        nc.gpsimd.dma_start(                          # SWDGE: casts f32->bf16
            cb_sb[:],
            cb_ext.rearrange("d p y -> p d y"),
        )
        ident_b = consts.tile([128, 128], BF16)
        make_identity(nc, ident_b[:])

        xbs = {}

        def load_frame(g):
            if g >= n_frames:
                return
            xb = xin.tile([128, 2, HW], BF16, tag="x")
            for t in range(2):
                nc.sync.dma_start(
                    xb[:, t, :], x_ext[g, t * 128:(t + 1) * 128, :])
            xbs[g] = xb

        load_frame(0)
        load_frame(1)

        for f in range(n_frames):
            load_frame(f + 2)
            xb = xbs.pop(f)

            # ---------- stats: sums on ACT (accum), max tree on DVE ------
            pr_max = small.tile([128, 2], F32, tag="prmax")
            pr_sum = small.tile([128, 2], F32, tag="prsum")
            scr = scrp.tile([128, 2, HW], BF16, tag="scr")
            for t in range(2):
                nc.scalar.activation(
                    scr[:, t, :], xb[:, t, :], ACTF.Copy,
                    accum_out=pr_sum[:, t:t + 1])
            f1 = foldp.tile([128, 2, HHW], BF16, tag="f1")
            nc.vector.tensor_tensor(
                out=f1[:], in0=xb[:, :, 0:HHW], in1=xb[:, :, HHW:HW],
                op=ALU.max)
            f2 = foldp.tile([128, 2, 784], BF16, tag="f2")
            nc.vector.tensor_tensor(
                out=f2[:], in0=f1[:, :, 0:784], in1=f1[:, :, 784:HHW],
                op=ALU.max)
            f3 = foldp.tile([128, 2, 392], BF16, tag="f3")
            nc.vector.tensor_tensor(
                out=f3[:], in0=f2[:, :, 0:392], in1=f2[:, :, 392:784],
                op=ALU.max)
            nc.vector.tensor_reduce(
                out=pr_max[:], in_=f3[:, :, :],
                axis=mybir.AxisListType.X, op=ALU.max)

            # ---------- MLP on PE ----------
            ph = pmp.tile([16, 2], F32, tag="misc")
            for si, prs in ((0, pr_max), (1, pr_sum)):
                for t in range(2):
                    nc.tensor.matmul(ph[:, si:si + 1], w1_sb[:, t, :],
                                     prs[:, t:t + 1],
                                     start=(t == 0), stop=(t == 1))
            h = small.tile([16, 2], F32, tag="h")
            nc.scalar.activation(h[:, 0:1], ph[:, 0:1], ACTF.Relu)
            nc.scalar.activation(h[:, 1:2], ph[:, 1:2], ACTF.Relu,
                                 scale=1.0 / HW)
            hs = small.tile([16, 1], F32, tag="hs")
            nc.vector.tensor_tensor(out=hs[:], in0=h[:, 0:1], in1=h[:, 1:2],
                                    op=ALU.add)
            pca = pmp.tile([128, 2], F32, tag="misc")
            for t in range(2):
                nc.tensor.matmul(pca[:, t:t + 1],
                                 w2_sb[:, t * 128:(t + 1) * 128], hs[:],
                                 start=True, stop=True)
            ca = small.tile([128, 2], F32, tag="ca")
            nc.scalar.activation(ca[:], pca[:], ACTF.Sigmoid)
            ca_b = small.tile([128, 2], BF16, tag="ca_b")
            nc.scalar.activation(ca_b[:], pca[:], ACTF.Sigmoid)

            # ---------- xc_t = ca_t * xb_t, m1 = max (DVE TT 2x) -------
            xc = xcp.tile([128, 2, HW], BF16, tag="xc")
            nc.scalar.activation(xc[:, 0, :], xb[:, 0, :], ACTF.Copy,
                                 scale=ca[:, 0:1])
            nc.vector.tensor_scalar(
                out=xc[:, 1, :], in0=xb[:, 1, :],
                scalar1=ca[:, 1:2], scalar2=None, op0=ALU.mult)
            m1 = m1p.tile([128, HW], BF16, tag="m1")
            nc.vector.tensor_tensor(
                out=m1[:], in0=xc[:, 0, :], in1=xc[:, 1, :], op=ALU.max)

            # ---------- ssum via PE: chunks at partitions {0,32,64} -----
            pss = pssp.tile([65, 3, 512], F32, tag="pss")
            for t in range(2):
                for j in range(7):
                    bp, bk = 32 * (j // 3), j % 3
                    nc.tensor.matmul(
                        pss[bp:bp + 1, bk, 0:SCHK],
                        ca_b[:, t:t + 1],
                        xb[:, t, j * SCHK:(j + 1) * SCHK],
                        start=(t == 0), stop=(t == 1))
            ssb = sap.tile([65, 3, SCHK], BF16, tag="ssb")
            nc.scalar.activation(ssb[:], pss[:, :, 0:SCHK], ACTF.Copy)

            # ---------- smax: transposes + 2 PSUM reduces ---------------
            sm_cols = small.tile([112, 28], BF16, tag="smc")
            pts = []
            for g in range(2):
                pt = ptp.tile([112, 14, 128], BF16, tag=f"pt{g}")
                for j in range(14):
                    c = g * 14 + j
                    nc.tensor.transpose(
                        pt[:, j, :], m1[:, c * TCHK:(c + 1) * TCHK],
                        ident_b[:])
                pts.append(pt)

            for g in range(2):
                nc.vector.tensor_reduce(
                    out=sm_cols[:, g * 14:(g + 1) * 14], in_=pts[g][:, :, :],
                    axis=mybir.AxisListType.X, op=ALU.max)
            psmT = pmp.tile([28, 112], BF16, tag="misc")
            nc.tensor.transpose(psmT[:], sm_cols[:], ident_b[0:112, 0:112])
            smb = sap.tile([28, 112], BF16, tag="smb")
            nc.scalar.activation(smb[:], psmT[:], ACTF.Copy)

            # ---------- conv input assembly (sbuf->sbuf DMAs) -----------
            s_pad = sap.tile([124, WP], BF16, tag="s_pad")
            nc.gpsimd.memset(s_pad[:], 0.0)
            # avg rows (c=0): partitions 3..58; pixel order (p, bank, n)
            nc.scalar.dma_start(s_pad[PAD:PAD + 48, PAD:PAD + W],
                                ssb[0:33:32, :, :])
            nc.scalar.dma_start(s_pad[PAD + 48:PAD + H, PAD:PAD + W],
                                ssb[64:65, 0, :])
            # max rows (c=1): partitions 65..120
            nc.sync.dma_start(s_pad[HP + PAD:HP + PAD + H, PAD:PAD + W],
                              smb[:])

            # ---------- conv: 7 accumulating matmuls ----------
            pcv = pmp.tile([H, W], F32, tag="misc")
            for dx in range(7):
                nc.tensor.matmul(pcv[:], cb_sb[:, dx, :],
                                 s_pad[:, dx:dx + W],
                                 start=(dx == 0), stop=(dx == 6))
            sa_yx = small.tile([H, W], BF16, tag="sa_yx")
            nc.scalar.activation(sa_yx[:], pcv[:], ACTF.Sigmoid)

            # ---------- sa broadcast ----------
            sa_row = sap.tile([1, HW], BF16, tag="sa_row")
            nc.sync.dma_start(sa_row[:], sa_yx[:])
            sab = sabp.tile([128, HW], BF16, tag="sab")
            nc.gpsimd.partition_broadcast(sab[:], sa_row[0:1, :],
                                          channels=128)

            # ---------- final: ob_t = xc_t * sab (TT 2x) ----
            ob = obp.tile([128, 2, HW], BF16, tag="ob")
            for t in range(2):
                nc.vector.tensor_tensor(
                    out=ob[:, t, :], in0=xc[:, t, :], in1=sab[:],
                    op=ALU.mult)
                eng = nc.gpsimd if t == 0 else nc.sync
                eng.dma_start(
                    out_ext[f, t * 128:(t + 1) * 128, :], ob[:, t, :])

    nc.finalize()
    return nc


_NC_CACHE = {}


def _get_nc(n_frames: int):
    if n_frames not in _NC_CACHE:
        _NC_CACHE[n_frames] = build_nc(n_frames)
    return _NC_CACHE[n_frames]


def _make_in_maps(f, w1, w2, conv_w):
    import ml_dtypes
    w1 = np.ascontiguousarray(np.asarray(w1, dtype=np.float32))
    w2 = np.ascontiguousarray(np.asarray(w2, dtype=np.float32))
    conv_w = np.asarray(conv_w, dtype=np.float32)
    convb = _build_conv_lhsT(conv_w)
    frames = np.asarray(f, dtype=np.float32).reshape(FRAMES, C, HW)
    frames = frames.astype(ml_dtypes.bfloat16)
    in_maps = []
    for i in range(N_CORES):
        in_maps.append({
            "x": np.ascontiguousarray(frames[i * FPC:(i + 1) * FPC]),
            "w1": w1,
            "w2": w2,
            "convb": convb,
        })
    return in_maps


def kernel(f: np.ndarray, w1: np.ndarray, w2: np.ndarray,
           conv_w: np.ndarray) -> np.ndarray:
    in_maps = _make_in_maps(f, w1, w2, conv_w)
    nc = _get_nc(FPC)
    res = run_bass_kernel_spmd(nc, in_maps, core_ids=list(range(N_CORES)))
    out = np.concatenate(
        [np.asarray(res.results[i]["out"]).astype(np.float32)
         for i in range(N_CORES)], axis=0)
    return out.reshape(B, T, C, H, W)


if __name__ == "__main__":
    rng = np.random.default_rng(0)
    f = rng.standard_normal((B, T, C, H, W), dtype=np.float32)
    w1 = rng.standard_normal((C, 16), dtype=np.float32) / 16.0
    w2 = rng.standard_normal((16, C), dtype=np.float32) / 4.0
    conv_w = rng.standard_normal((1, 2, 7, 7), dtype=np.float32) * 0.1
    out = kernel(f, w1, w2, conv_w)
    print("kernel ran, out shape", out.shape, out.dtype)


# revision 44
# speedup vs baseline: 1.0084x; 1.0084x over previous
"""CBAM (channel + spatial attention) Trainium2 kernel, 8-core data parallel.

Problem: f [8, 8, 256, 56, 56] f32 -> out same shape.
  x = f.reshape(BT, C, H, W)
  ca = sigmoid(mlp(max_hw(x)) + mlp(mean_hw(x)));  xc = ca * x
  s  = conv7x7([mean_c(xc); max_c(xc)]);           out = sigmoid(s) * xc

Strategy (per NeuronCore, 8 frames each, no collectives):
  - bf16 end-to-end on device: host casts f to bf16, device returns bf16,
    host upcasts -> halves HBM traffic (rel-err budget 2e-2 >> bf16 noise)
  - loads on SP HWDGE prefetched 2 frames ahead of the frame body so
    mid-chain DMAs never head-of-line-block the next frame's load
  - channel-sum: ACT activation accum_out per tile (f32 accumulator; DVE
    reduce-add accumulates in bf16 on HW and is NOT accurate enough)
  - channel-max: DVE TT fold tree L1-L3 (bf16 2x) + small 1x reduce
  - xc0 via ACT scaled copy (per-partition scale AP), xc1 via DVE
    tensor_scalar (4x); m1 = max(xc0, xc1) via DVE TT (2x)
  - ssum: PE matmuls into PSUM chunks at partitions {0,32,64} x 3 banks
    (matmul PSUM base partition must be 0/32/64), single ACT drain,
    partition-strided DMA scatters rows 0/32/64 into the conv input
  - smax: 28 PE transposes -> PSUM bf16 (2 groups), 2 direct DVE
    reduces (PSUM-in 1x, bf16 out), transpose back, ACT drain
  - conv 7x7 as 7 accumulating PE matmuls (banded lhsT, 1/C folded in)
  - sa broadcast to 128 partitions via GPSIMD partition_broadcast ucode
    (DMA stride-0 broadcast serializes on the source partition; don't)
  - final: ob_t = xc_t * sab via DVE TT (2x); ob emitted one frame late
    (software pipelining); store t0 on GPSIMD SWDGE, t1 on SP HWDGE
"""

import sys
from contextlib import ExitStack

import numpy as np

if "/opt/trn_rl_repo" not in sys.path:
    sys.path.insert(0, "/opt/trn_rl_repo")

import concourse.bass as bass
import concourse.tile as tile
from concourse import bacc, mybir
from concourse.bass_utils import run_bass_kernel_spmd
from concourse.masks import make_identity

F32 = mybir.dt.float32
BF16 = mybir.dt.bfloat16
ALU = mybir.AluOpType
ACTF = mybir.ActivationFunctionType

N_CORES = 8
B, T, C, H, W = 8, 8, 256, 56, 56
HW = H * W            # 3136
FRAMES = B * T        # 64
FPC = FRAMES // N_CORES  # frames per core = 8
PAD = 3
HP, WP = H + 2 * PAD, W + 2 * PAD  # 62, 62
SCHK = 448            # ssum chunk width (7 chunks)
TCHK = 112            # transpose chunk (28 chunks, 2 groups of 14)
HHW = HW // 2         # 1568


def _build_conv_lhsT(conv_w: np.ndarray) -> np.ndarray:
    """Banded matrices for the 7x7 conv as 7 accumulating matmuls over y.

    B[dx] : [124, 56], rows = c*62 + y_in, cols = y_out.
    B[dx][c*62 + yi, yo] = w_eff[c, yi-yo, dx] for 0 <= yi-yo <= 6.
    The channel-mean 1/C is folded into the avg branch (c=0).
    """
    w_eff = conv_w[0].astype(np.float64).copy()  # [2, 7, 7]
    w_eff[0] /= C
    Bm = np.zeros((7, 2 * HP, H), dtype=np.float32)
    dyi = np.arange(7)
    for dx in range(7):
        for c in range(2):
            for yo in range(H):
                Bm[dx, c * HP + yo + dyi, yo] = w_eff[c, :, dx]
    return Bm


def build_nc(n_frames: int = FPC):
    nc = bacc.Bacc("TRN2", target_bir_lowering=False, debug=False,
                   num_devices=N_CORES)

    x_ext = nc.dram_tensor("x", [n_frames, C, HW], BF16, kind="ExternalInput")
    w1_ext = nc.dram_tensor("w1", [C, 16], F32, kind="ExternalInput")
    w2_ext = nc.dram_tensor("w2", [16, C], F32, kind="ExternalInput")
    cb_ext = nc.dram_tensor("convb", [7, 2 * HP, H], F32, kind="ExternalInput")
    out_ext = nc.dram_tensor("out", [n_frames, C, HW], BF16,
                             kind="ExternalOutput")

    with tile.TileContext(nc) as tc, ExitStack() as ctx:
        consts = ctx.enter_context(tc.tile_pool(name="consts", bufs=1))
        xin = ctx.enter_context(tc.tile_pool(name="xin", bufs=3))
        scrp = ctx.enter_context(tc.tile_pool(name="scr", bufs=1))
        foldp = ctx.enter_context(tc.tile_pool(name="fold", bufs=1))
        xcp = ctx.enter_context(tc.tile_pool(name="xc", bufs=2))
        m1p = ctx.enter_context(tc.tile_pool(name="m1", bufs=2))
        sabp = ctx.enter_context(tc.tile_pool(name="sab", bufs=2))
        obp = ctx.enter_context(tc.tile_pool(name="ob", bufs=2))
        sap = ctx.enter_context(tc.tile_pool(name="sa", bufs=2))
        small = ctx.enter_context(tc.tile_pool(name="small", bufs=3))
        # PSUM: pss 3 banks + pt 2x2 banks + misc 1 bank = 8
        pssp = ctx.enter_context(tc.tile_pool(name="pss", bufs=1, space="PSUM"))
        ptp = ctx.enter_context(tc.tile_pool(name="pt", bufs=1, space="PSUM"))
        pmp = ctx.enter_context(tc.tile_pool(name="pm", bufs=1, space="PSUM"))

        xbs = {}

        def load_frame(g):
            if g >= n_frames:
                return
            xb = xin.tile([128, 2, HW], BF16, tag="x")
            for t in range(2):
                nc.sync.dma_start(
                    xb[:, t, :], x_ext[g, t * 128:(t + 1) * 128, :])
            xbs[g] = xb

        # first frames load before the weights: weights are not needed
        # until MLP(0), so they must not delay pipeline fill on the queue
        load_frame(0)
        load_frame(1)

        # ---- constants / weights (loaded once) ----
        w1_sb = consts.tile([128, 2, 16], F32)       # [k, ktile, m]
        for t in range(2):
            nc.sync.dma_start(w1_sb[:, t, :], w1_ext[t * 128:(t + 1) * 128, :])
        w2_sb = consts.tile([16, C], F32)
        nc.sync.dma_start(w2_sb[:], w2_ext[:, :])
        cb_sb = consts.tile([124, 7, H], BF16)       # [y_in(+c), dx, y_out]
        nc.gpsimd.dma_start(                          # SWDGE: casts f32->bf16
            cb_sb[:],
            cb_ext.rearrange("d p y -> p d y"),
        )
        ident_b = consts.tile([128, 128], BF16)
        make_identity(nc, ident_b[:])

        for f in range(n_frames):
            load_frame(f + 2)
            xb = xbs.pop(f)

            # ---------- stats: sums on ACT (accum), max tree on DVE ------
            pr_max = small.tile([128, 2], F32, tag="prmax")
            pr_sum = small.tile([128, 2], F32, tag="prsum")
            scr = scrp.tile([128, 2, HW], BF16, tag="scr")
            for t in range(2):
                nc.scalar.activation(
                    scr[:, t, :], xb[:, t, :], ACTF.Copy,
                    accum_out=pr_sum[:, t:t + 1])
            f1 = foldp.tile([128, 2, HHW], BF16, tag="f1")
            nc.vector.tensor_tensor(
                out=f1[:], in0=xb[:, :, 0:HHW], in1=xb[:, :, HHW:HW],
                op=ALU.max)
            f2 = foldp.tile([128, 2, 784], BF16, tag="f2")
            nc.vector.tensor_tensor(
                out=f2[:], in0=f1[:, :, 0:784], in1=f1[:, :, 784:HHW],
                op=ALU.max)
            f3 = foldp.tile([128, 2, 392], BF16, tag="f3")
            nc.vector.tensor_tensor(
                out=f3[:], in0=f2[:, :, 0:392], in1=f2[:, :, 392:784],
                op=ALU.max)
            nc.vector.tensor_reduce(
                out=pr_max[:], in_=f3[:, :, :],
                axis=mybir.AxisListType.X, op=ALU.max)

            # ---------- MLP on PE ----------
            ph = pmp.tile([16, 2], F32, tag="misc")
            for si, prs in ((0, pr_max), (1, pr_sum)):
                for t in range(2):
                    nc.tensor.matmul(ph[:, si:si + 1], w1_sb[:, t, :],
                                     prs[:, t:t + 1],
                                     start=(t == 0), stop=(t == 1))
            h = small.tile([16, 2], F32, tag="h")
            nc.scalar.activation(h[:, 0:1], ph[:, 0:1], ACTF.Relu)
            nc.scalar.activation(h[:, 1:2], ph[:, 1:2], ACTF.Relu,
                                 scale=1.0 / HW)
            pca = pmp.tile([128, 2], F32, tag="misc")
            for t in range(2):
                nc.tensor.matmul(pca[:, t:t + 1],
                                 w2_sb[:, t * 128:(t + 1) * 128], h[:, 0:1],
                                 start=True, stop=False)
                nc.tensor.matmul(pca[:, t:t + 1],
                                 w2_sb[:, t * 128:(t + 1) * 128], h[:, 1:2],
                                 start=False, stop=True)
            ca = small.tile([128, 2], F32, tag="ca")
            nc.scalar.activation(ca[:], pca[:], ACTF.Sigmoid)
            ca_b = small.tile([128, 2], BF16, tag="ca_b")
            nc.scalar.activation(ca_b[:], pca[:], ACTF.Sigmoid)

            # ---------- xc_t = ca_t * xb_t, m1 = max (DVE TT 2x) -------
            xc = xcp.tile([128, 2, HW], BF16, tag="xc")
            nc.scalar.activation(xc[:, 0, :], xb[:, 0, :], ACTF.Copy,
                                 scale=ca[:, 0:1])
            nc.vector.tensor_scalar(
                out=xc[:, 1, :], in0=xb[:, 1, :],
                scalar1=ca[:, 1:2], scalar2=None, op0=ALU.mult)
            m1 = m1p.tile([128, HW], BF16, tag="m1")
            nc.vector.tensor_tensor(
                out=m1[:], in0=xc[:, 0, :], in1=xc[:, 1, :], op=ALU.max)

            # ---------- ssum via PE: chunks at partitions {0,32,64} -----
            pss = pssp.tile([65, 3, 512], F32, tag="pss")
            for t in range(2):
                for j in range(7):
                    bp, bk = 32 * (j // 3), j % 3
                    nc.tensor.matmul(
                        pss[bp:bp + 1, bk, 0:SCHK],
                        ca_b[:, t:t + 1],
                        xb[:, t, j * SCHK:(j + 1) * SCHK],
                        start=(t == 0), stop=(t == 1))
            ssb = sap.tile([65, 3, SCHK], BF16, tag="ssb")
            nc.scalar.activation(ssb[:], pss[:, :, 0:SCHK], ACTF.Copy)

            # ---------- smax: transposes + 2 PSUM reduces ---------------
            sm_cols = small.tile([112, 28], BF16, tag="smc")
            pts = []
            for g in range(2):
                pt = ptp.tile([112, 14, 128], BF16, tag=f"pt{g}")
                for j in range(14):
                    c = g * 14 + j
                    nc.tensor.transpose(
                        pt[:, j, :], m1[:, c * TCHK:(c + 1) * TCHK],
                        ident_b[:])
                pts.append(pt)

            for g in range(2):
                nc.vector.tensor_reduce(
                    out=sm_cols[:, g * 14:(g + 1) * 14], in_=pts[g][:, :, :],
                    axis=mybir.AxisListType.X, op=ALU.max)
            psmT = pmp.tile([28, 112], BF16, tag="misc")
            nc.tensor.transpose(psmT[:], sm_cols[:], ident_b[0:112, 0:112])
            smb = sap.tile([28, 112], BF16, tag="smb")
            nc.scalar.activation(smb[:], psmT[:], ACTF.Copy)

            # ---------- conv input assembly (sbuf->sbuf DMAs) -----------
            s_pad = sap.tile([124, WP], BF16, tag="s_pad")
            nc.gpsimd.memset(s_pad[:], 0.0)
            # avg rows (c=0): partitions 3..58; pixel order (p, bank, n)
            nc.scalar.dma_start(s_pad[PAD:PAD + 48, PAD:PAD + W],
                                ssb[0:33:32, :, :])
            nc.scalar.dma_start(s_pad[PAD + 48:PAD + H, PAD:PAD + W],
                                ssb[64:65, 0, :])
            # max rows (c=1): partitions 65..120
            nc.sync.dma_start(s_pad[HP + PAD:HP + PAD + H, PAD:PAD + W],
                              smb[:])

            # ---------- conv: 7 accumulating matmuls ----------
            pcv = pmp.tile([H, W], F32, tag="misc")
            for dx in range(7):
                nc.tensor.matmul(pcv[:], cb_sb[:, dx, :],
                                 s_pad[:, dx:dx + W],
                                 start=(dx == 0), stop=(dx == 6))
            sa_yx = small.tile([H, W], BF16, tag="sa_yx")
            nc.scalar.activation(sa_yx[:], pcv[:], ACTF.Sigmoid)

            # ---------- sa broadcast ----------
            sa_row = sap.tile([1, HW], BF16, tag="sa_row")
            nc.sync.dma_start(sa_row[:], sa_yx[:])
            sab = sabp.tile([128, HW], BF16, tag="sab")
            nc.gpsimd.partition_broadcast(sab[:], sa_row[0:1, :],
                                          channels=128)

            # ---------- final: ob_t = xc_t * sab (TT 2x) ----
            ob = obp.tile([128, 2, HW], BF16, tag="ob")
            for t in range(2):
                nc.vector.tensor_tensor(
                    out=ob[:, t, :], in0=xc[:, t, :], in1=sab[:],
                    op=ALU.mult)
                eng = nc.gpsimd if t == 0 else nc.sync
                eng.dma_start(
                    out_ext[f, t * 128:(t + 1) * 128, :], ob[:, t, :])

    nc.finalize()
    return nc


_NC_CACHE = {}


def _get_nc(n_frames: int):
    if n_frames not in _NC_CACHE:
        _NC_CACHE[n_frames] = build_nc(n_frames)
    return _NC_CACHE[n_frames]


def _make_in_maps(f, w1, w2, conv_w):
    import ml_dtypes
    w1 = np.ascontiguousarray(np.asarray(w1, dtype=np.float32))
    w2 = np.ascontiguousarray(np.asarray(w2, dtype=np.float32))
    conv_w = np.asarray(conv_w, dtype=np.float32)
    convb = _build_conv_lhsT(conv_w)
    frames = np.asarray(f, dtype=np.float32).reshape(FRAMES, C, HW)
    frames = frames.astype(ml_dtypes.bfloat16)
    in_maps = []
    for i in range(N_CORES):
        in_maps.append({
            "x": np.ascontiguousarray(frames[i * FPC:(i + 1) * FPC]),
            "w1": w1,
            "w2": w2,
            "convb": convb,
        })
    return in_maps


def kernel(f: np.ndarray, w1: np.ndarray, w2: np.ndarray,
           conv_w: np.ndarray) -> np.ndarray:
    in_maps = _make_in_maps(f, w1, w2, conv_w)
    nc = _get_nc(FPC)
    res = run_bass_kernel_spmd(nc, in_maps, core_ids=list(range(N_CORES)))
    out = np.concatenate(
        [np.asarray(res.results[i]["out"]).astype(np.float32)
         for i in range(N_CORES)], axis=0)
    return out.reshape(B, T, C, H, W)


if __name__ == "__main__":
    rng = np.random.default_rng(0)
    f = rng.standard_normal((B, T, C, H, W), dtype=np.float32)
    w1 = rng.standard_normal((C, 16), dtype=np.float32) / 16.0
    w2 = rng.standard_normal((16, C), dtype=np.float32) / 4.0
    conv_w = rng.standard_normal((1, 2, 7, 7), dtype=np.float32) * 0.1
    out = kernel(f, w1, w2, conv_w)
    print("kernel ran, out shape", out.shape, out.dtype)
